# revision 6
# baseline (speedup 1.0000x reference)
"""Trainium2 Bass kernel for nn_AttentiveStateMLP (B=65536).

Strategy: pure data-parallel over 8 NeuronCores (8192 samples/core).
Everything stays FEATURE-major (features on partitions, samples in the
free dim) end-to-end — no transposes anywhere:
  - enc = relu(W1^T x + b1) via fp32 matmuls + ACT bias-relu
  - q/k/v/tok per token fused from enc (P_t folded into Wqkv) via bf16
    matmuls, biases applied by per-partition ACT bias on the PSUM->SBUF
    cast to bf16
  - scores: one bf16 broadcast product per 64-sample chunk, then the
    per-head d-reduction as a PE matmul whose lhsT both sums d and
    REPLICATES each head's score across that head's 32 partitions
  - softmax over k without max-subtraction (scores are in [-0.5, 0.4])
  - ctx: bf16 product with v + innermost-k reduce
  - attn_out = Wo^T ctx via fp32 matmul; h = attn_out + tok (bo folded
    into tok's bias)
  - LayerNorm stats via ones-matmuls (mean and E[h^2] replicated across
    partitions by an all-ones/128 lhsT); istd = Rsqrt(var + eps)
  - pooled projection: m = sum_t istd_t*h_t; the sum_t istd_t*mu_t
    correction folds into the weights (s_im = colmean(m)), so
    out = relu(m @ Wp6'), computed as fp32 matmuls + ACT relu-bias
  - final projection uses the m-block as the stationary operand so\n    the output lands sample-major; output DMAs are contiguous stores
"""
import numpy as np

B = 65536
NCORES = 8
BL = B // NCORES          # 8192 samples per core
E = 128
NH, DH = 4, 32
OUT = 256
LN_EPS = 1e-5

C32 = 1184                # fp32 const blob cols
C16 = 3712                # bf16 const blob cols
NC_CH = 64                # samples per attention chunk (PSUM-limited)

# cb32 column layout
_W1 = 0            # [29 rows, 384]
_B1T = 384         # [128, 3]
_BQ = 387          # [128, 6]
_BK = 393
_BV = 399
_BTOK = 405
_BP1 = 411         # [128, 2]
_EPS = 413         # [128, 1]
_WO = 414          # [128, 128] fp32
_WP = 542          # [128, 256] fp32 (Wp6')
_BP1R = 800        # [1, 256] fp32 (bp1 as a row, for the K=1 bias matmul)
_ONE1 = 1056       # [1, 128] fp32 ones (K=1 bias matmul lhsT)
# cb16 column layout
_QW = 0            # [*, 6*128]
_KW = 768
_VW = 1536
_TW = 2304
_HMASK = 3072      # [128, 128]
_ONES = 3200       # [128, 128] = 1/128
# 3328..3712 spare

_PROGRAM = None
_RUNNER = None

# token -> (enc chunk index, row range within chunk)
SEG = [(0, 0, 64), (0, 64, 128), (1, 0, 32), (1, 32, 64), (1, 64, 128),
       (2, 0, 128)]


def _build_program(bl=BL, pad=False):
    from contextlib import ExitStack
    import concourse.bass as bass
    import concourse.tile as tile
    from concourse import mybir

    F32 = mybir.dt.float32
    BF16 = mybir.dt.bfloat16
    AF = mybir.ActivationFunctionType
    OP = mybir.AluOpType
    AX = mybir.AxisListType

    nst = bl // 512

    nc = bass.Bass()
    xt_d = nc.dram_tensor("xt", [29, bl], F32, kind="ExternalInput")
    cb32_d = nc.dram_tensor("cb32", [128, C32], F32, kind="ExternalInput")
    cb16_d = nc.dram_tensor("cb16", [128, C16], BF16, kind="ExternalInput")
    out_d = nc.dram_tensor("out", [bl, 256], F32, kind="ExternalOutput")
    pad_d = nc.dram_tensor("pad", [bl, 256], F32,
                           kind="ExternalInput") if pad else None

    with nc.allow_low_precision("bf16 kernel, tol 2e-2"), \
            tile.TileContext(nc) as tc, ExitStack() as ctx:
        consts = ctx.enter_context(tc.tile_pool(name="consts", bufs=1))
        sb = ctx.enter_context(tc.tile_pool(name="sb", bufs=1))
        sb2 = ctx.enter_context(tc.tile_pool(name="sb2", bufs=2))
        mmps = ctx.enter_context(tc.tile_pool(name="mmps", bufs=2, space="PSUM"))
        scps = ctx.enter_context(tc.tile_pool(name="scps", bufs=1, space="PSUM"))

        # constants; DVE-shield the DMA-landed blobs before matmuls touch them
        cb32r = consts.tile([128, C32], F32)
        nc.sync.dma_start(cb32r, cb32_d[:, :])
        cb32 = consts.tile([128, C32], F32)
        nc.vector.tensor_copy(cb32, cb32r)
        cb16r = consts.tile([128, C16], BF16)
        nc.sync.dma_start(cb16r, cb16_d[:, :])
        cb16 = consts.tile([128, C16], BF16)
        nc.vector.tensor_copy(cb16, cb16r)

        w1 = cb32[0:29, _W1:_W1 + 384]
        b1t = cb32[:, _B1T:_B1T + 3]
        eps_c = cb32[:, _EPS:_EPS + 1]
        wo = cb32[:, _WO:_WO + 128]
        wp = cb32[:, _WP:_WP + 256]
        hmask = cb16[:, _HMASK:_HMASK + 128]
        ones = cb16[:, _ONES:_ONES + 128]

        xt_all = consts.tile([29, bl], F32)
        nc.sync.dma_start(xt_all, xt_d[:, :])
        if pad_d is not None:
            # timing-only variant: anchor the pad input with a tiny read so
            # its host->device transfer matches the baseline program's
            padt = consts.tile([1, 256], F32)
            nc.sync.dma_start(padt, pad_d[0:1, :])

        for st in range(nst):
            xs = xt_all[:, st * 512:(st + 1) * 512]

            # ---- P1: encoders ----
            enc16 = sb.tile([128, 3, 512], BF16, tag="enc")
            for i in range(3):
                ps = mmps.tile([128, 512], F32, tag="mm")
                nc.tensor.matmul(ps, lhsT=w1[:, i * 128:(i + 1) * 128],
                                 rhs=xs, start=True, stop=True)
                nc.scalar.activation(out=enc16[:, i, :], in_=ps, func=AF.Relu,
                                     bias=b1t[:, i:i + 1], scale=1.0)

            # ---- P2: q/k/v/tok per token (P folded into Wqkv) ----
            q16 = sb.tile([128, 6, 512], BF16, tag="q16")
            k16 = sb.tile([128, 6, 512], BF16, tag="k16")
            v16 = sb.tile([128, 6, 512], BF16, tag="v16")
            tok16 = sb.tile([128, 6, 512], BF16, tag="tok16")
            for t in range(6):
                ech, r0, r1 = SEG[t]
                rhs = enc16[r0:r1, ech, :]
                for (wc, bc, dst) in ((_QW, _BQ, q16), (_KW, _BK, k16),
                                      (_VW, _BV, v16), (_TW, _BTOK, tok16)):
                    ps = mmps.tile([128, 512], F32, tag="mm")
                    nc.tensor.matmul(ps, lhsT=cb16[r0:r1, wc + 128 * t:wc + 128 * (t + 1)],
                                     rhs=rhs, start=True, stop=True)
                    nc.scalar.activation(out=dst[:, t, :], in_=ps, func=AF.Identity,
                                         bias=cb32[:, bc + t:bc + t + 1], scale=1.0)

            # ---- P3/P4: attention per 64-sample chunk ----
            # layout (a, b, s): stride-1 innermost on every TT operand -> 2x DVE
            ctx32 = sb.tile([128, 6, 512], F32, tag="ctx32")
            ssum32 = sb.tile([128, 6, 512], F32, tag="ssum32")
            nch = 512 // NC_CH
            for c in range(nch):
                sl = slice(c * NC_CH, (c + 1) * NC_CH)
                # prod[p, a, b, s] = q[p, a, s] * k[p, b, s]
                prod16 = sb2.tile([128, 6, 6, NC_CH], BF16, tag="prod")
                qv = q16[:, :, sl].unsqueeze(2).broadcast_to([128, 6, 6, NC_CH])
                kv = k16[:, :, sl].unsqueeze(1).broadcast_to([128, 6, 6, NC_CH])
                nc.vector.tensor_tensor(out=prod16, in0=qv, in1=kv, op=OP.mult)
                # d-reduce + head-replicate on PE: sc[(h,d'), (b,s)] per a
                sc = scps.tile([128, 6, 512], F32, tag="sc")
                for a in range(6):
                    nc.tensor.matmul(sc[:, a, 0:NC_CH * 6], lhsT=hmask,
                                     rhs=prod16[:, a, :, :], start=True, stop=True)
                # exp (no max-subtraction; scores are tiny), sum over b
                esc16 = sb2.tile([128, 6, 6, NC_CH], BF16, tag="esc")
                nc.scalar.activation(out=esc16.rearrange("p a b s -> p a (b s)"),
                                     in_=sc[:, :, 0:NC_CH * 6], func=AF.Exp)
                nc.vector.tensor_reduce(out=ssum32[:, :, sl],
                                        in_=esc16.rearrange("p a b s -> p a s b"),
                                        axis=AX.X, op=OP.add)
                # ctx_un[p, a, s] = sum_b esc[p, a, b, s] * v[p, b, s]
                prod2 = sb2.tile([128, 6, 6, NC_CH], BF16, tag="prod2")
                vv = v16[:, :, sl].unsqueeze(1).broadcast_to([128, 6, 6, NC_CH])
                nc.vector.tensor_tensor(out=prod2, in0=esc16, in1=vv, op=OP.mult)
                nc.vector.tensor_reduce(out=ctx32[:, :, sl],
                                        in_=prod2.rearrange("p a b s -> p a s b"),
                                        axis=AX.X, op=OP.add)
            # normalize once per supertile: ctx = ctx_un * 1/ssum (the
            # replicated-score layout makes ssum shape-compatible with ctx)
            rsum32 = sb.tile([128, 6, 512], F32, tag="rsum32")
            nc.vector.reciprocal(out=rsum32.rearrange("p a s -> p (a s)"),
                                 in_=ssum32.rearrange("p a s -> p (a s)"))
            nc.vector.tensor_tensor(out=ctx32.rearrange("p a s -> p (a s)"),
                                    in0=ctx32.rearrange("p a s -> p (a s)"),
                                    in1=rsum32.rearrange("p a s -> p (a s)"),
                                    op=OP.mult)

            # ---- P5: Wo, residual, LayerNorm, pooled projection ----
            w16 = sb.tile([128, 6, 512], BF16, tag="w16")
            for a in range(6):
                ps = mmps.tile([128, 512], F32, tag="mm")
                nc.tensor.matmul(ps, lhsT=wo,
                                 rhs=ctx32[:, a, :],
                                 start=True, stop=True)
                nc.scalar.activation(out=w16[:, a, :], in_=ps, func=AF.Copy)
            h16 = sb.tile([128, 6, 512], BF16, tag="h16")
            nc.vector.tensor_tensor(
                out=h16.rearrange("p a s -> p (a s)"),
                in0=w16.rearrange("p a s -> p (a s)"),
                in1=tok16.rearrange("p a s -> p (a s)"), op=OP.add)
            sq16 = sb.tile([128, 6, 512], BF16, tag="sq16")
            nc.vector.tensor_tensor(
                out=sq16.rearrange("p a s -> p (a s)"),
                in0=h16.rearrange("p a s -> p (a s)"),
                in1=h16.rearrange("p a s -> p (a s)"), op=OP.mult)
            musq16 = sb.tile([128, 6, 512], BF16, tag="musq")
            eh216 = sb.tile([128, 6, 512], BF16, tag="eh2")
            for a in range(6):
                ps = mmps.tile([128, 512], F32, tag="mm")
                nc.tensor.matmul(ps, lhsT=ones, rhs=h16[:, a, :],
                                 start=True, stop=True)
                nc.scalar.activation(out=musq16[:, a, :], in_=ps, func=AF.Square)
                ps2 = mmps.tile([128, 512], F32, tag="mm")
                nc.tensor.matmul(ps2, lhsT=ones, rhs=sq16[:, a, :],
                                 start=True, stop=True)
                nc.scalar.activation(out=eh216[:, a, :], in_=ps2, func=AF.Copy)
            var16 = sb.tile([128, 6, 512], BF16, tag="var16")
            nc.vector.tensor_tensor(
                out=var16.rearrange("p a s -> p (a s)"),
                in0=eh216.rearrange("p a s -> p (a s)"),
                in1=musq16.rearrange("p a s -> p (a s)"), op=OP.subtract)
            std16 = sb.tile([128, 6, 512], BF16, tag="std16")
            nc.scalar.activation(out=std16.rearrange("p a s -> p (a s)"),
                                 in_=var16.rearrange("p a s -> p (a s)"),
                                 func=AF.Sqrt, bias=eps_c, scale=1.0)
            istd16 = sb.tile([128, 6, 512], BF16, tag="istd16")
            nc.vector.reciprocal(out=istd16.rearrange("p a s -> p (a s)"),
                                 in_=std16.rearrange("p a s -> p (a s)"))
            # m[p, s] = sum_a h[p, a, s] * istd[p, a, s]   (s-major for reduce)
            mprod16 = sb.tile([128, 512, 6], BF16, tag="mprod")
            nc.vector.tensor_tensor(out=mprod16,
                                    in0=h16.rearrange("p a s -> p s a"),
                                    in1=istd16.rearrange("p a s -> p s a"),
                                    op=OP.mult)
            m32 = sb.tile([128, 512], F32, tag="m32")
            nc.vector.tensor_reduce(out=m32, in_=mprod16, axis=AX.X, op=OP.add)
            # out = relu(m @ Wp6' + bp1), emitted SAMPLE-major directly:
            # lhsT = a 128-sample block of m, rhs = Wp6' -> psum [s, 256];
            # bp1 added by a K=1 accumulating matmul; contiguous DMA out.
            for blk in range(4):
                s0 = st * 512 + blk * 128
                ps = mmps.tile([128, 256], F32, tag="mm")
                nc.tensor.matmul(ps, lhsT=m32[:, blk * 128:(blk + 1) * 128],
                                 rhs=wp, start=True, stop=False)
                nc.tensor.matmul(ps, lhsT=cb32[0:1, _ONE1:_ONE1 + 128],
                                 rhs=cb32[0:1, _BP1R:_BP1R + 256],
                                 start=False, stop=True)
                o32 = sb2.tile([128, 256], F32, tag="o32")
                nc.scalar.activation(out=o32, in_=ps, func=AF.Relu)
                nc.sync.dma_start(out_d[s0:s0 + 128, :], o32)

    return nc


def _legalize_waits(nc):
    """This container's walrus accepts at most 1 sync wait per instruction
    (2 on EventSemaphore). Tile emits more. Split the excess onto
    same-engine EventSemaphore nops inserted before the instruction."""
    from concourse import mybir
    n_new = 0
    for fn in nc.m.functions:
        for blk in fn.blocks:
            insts = blk.instructions
            out = []
            for inst in insts:
                si = inst.sync_info
                cap = 2 if isinstance(inst, mybir.InstEventSemaphore) else 1
                if si is not None and si.on_wait is not None and len(si.on_wait) > cap:
                    waits = list(si.on_wait)
                    keep = waits[:cap]
                    extra = waits[cap:]
                    for j in range(0, len(extra), 2):
                        chunk = extra[j:j + 2]
                        nop = mybir.InstEventSemaphore(
                            name=f"EVW-{n_new}",
                            engine=inst.engine,
                            ins=[], outs=[],
                            sync_info=mybir.SyncInfo(on_wait=chunk, on_update=[]),
                        )
                        n_new += 1
                        out.append(nop)
                    inst.sync_info = mybir.SyncInfo(
                        on_wait=keep, on_update=list(si.on_update or []))
                out.append(inst)
            if len(out) != len(insts):
                blk.instructions = out
    return n_new


def _host_prep(inputs):
    from concourse import mybir
    bf16 = mybir.dt.np(mybir.dt.bfloat16)
    f = np.float32
    x = np.asarray(inputs["x"], f)
    rs = f(1.0 / np.sqrt(DH))

    # block-diagonal combined encoder
    W1 = np.zeros((29, 384), f)
    b1 = np.zeros(384, f)
    enc_specs = [("Wv", "bv", 0, 3, 0, 64), ("Wm", "bm", 3, 8, 64, 128),
                 ("Wi", "bi", 8, 10, 128, 160), ("Wb", "bb", 10, 13, 160, 192),
                 ("Wc", "bc", 13, 19, 192, 256), ("Wf", "bf", 19, 29, 256, 384)]
    for wn, bn, r0, r1, c0, c1 in enc_specs:
        W1[r0:r1, c0:c1] = inputs[wn]
        b1[c0:c1] = inputs[bn]
    b1t = np.ascontiguousarray(b1.reshape(3, 128).T)  # [128, 3]

    P_list = [np.asarray(inputs[n], f) for n in ("Pv", "Pm", "Pi", "Pb", "Pc", "Pf")]
    p_list = [np.asarray(inputs[n], f) for n in ("pv", "pm", "pi", "pb", "pc", "pf")]

    Wqkv, bqkv = np.asarray(inputs["Wqkv"], f), np.asarray(inputs["bqkv"], f)
    Wq = Wqkv[:, 0:E] * rs
    Wk = Wqkv[:, E:2 * E]
    Wv_ = Wqkv[:, 2 * E:3 * E]
    bq = bqkv[0:E] * rs
    bk = bqkv[E:2 * E]
    bv = bqkv[2 * E:3 * E]

    Wo, bo = np.asarray(inputs["Wo"], f), np.asarray(inputs["bo"], f)
    g, beta = np.asarray(inputs["g"], f), np.asarray(inputs["beta"], f)
    Wp, bp = np.asarray(inputs["Wp"], f), np.asarray(inputs["bp"], f)
    Wp6 = Wp * g[:, None] / 6.0
    bp1 = (bp + beta @ Wp).astype(f)
    wpc6 = Wp6.sum(axis=0)
    Wp6p = (Wp6 - np.ones((128, 1), f) * wpc6[None, :] / 128.0).astype(f)

    cb32 = np.zeros((128, C32), f)
    cb32[0:29, _W1:_W1 + 384] = W1
    cb32[:, _B1T:_B1T + 3] = b1t
    cb32[:, _EPS] = LN_EPS
    cb32[:, _WO:_WO + 128] = Wo
    cb32[:, _WP:_WP + 256] = Wp6p

    cb16 = np.zeros((128, C16), np.float32)
    for t in range(6):
        ech, r0, r1 = SEG[t]
        P_t, p_t = P_list[t], p_list[t]
        cb16[r0:r1, _QW + 128 * t:_QW + 128 * (t + 1)] = P_t @ Wq
        cb16[r0:r1, _KW + 128 * t:_KW + 128 * (t + 1)] = P_t @ Wk
        cb16[r0:r1, _VW + 128 * t:_VW + 128 * (t + 1)] = P_t @ Wv_
        cb16[r0:r1, _TW + 128 * t:_TW + 128 * (t + 1)] = P_t
        cb32[:, _BQ + t] = p_t @ Wq + bq
        cb32[:, _BK + t] = p_t @ Wk + bk
        cb32[:, _BV + t] = p_t @ Wv_ + bv
    # tok bias must carry bo (residual: h = tok + ctx@Wo + bo)
    for t in range(6):
        cb32[:, _BTOK + t] = p_list[t] + bo
    cb32[:, _BP1 + 0] = bp1[0:128]
    cb32[:, _BP1 + 1] = bp1[128:256]
    cb32[0, _BP1R:_BP1R + 256] = bp1
    cb32[0, _ONE1:_ONE1 + 128] = 1.0

    hm = np.zeros((128, 128), np.float32)
    for h in range(NH):
        hm[h * DH:(h + 1) * DH, h * DH:(h + 1) * DH] = 1.0
    cb16[:, _HMASK:_HMASK + 128] = hm
    cb16[:, _ONES:_ONES + 128] = 1.0 / 128.0

    xt = np.ascontiguousarray(x.T)  # [29, B]
    return xt, {"cb32": cb32, "cb16": cb16.astype(bf16)}


def _make_runner(nc, ncores=NCORES):
    import jax
    from jax.sharding import Mesh, PartitionSpec
    from jax.experimental.shard_map import shard_map
    from concourse import mybir
    from concourse.bass2jax import (_bass_exec_p, install_neuronx_cc_hook,
                                    partition_id_tensor)

    install_neuronx_cc_hook()
    part_name = nc.partition_id_tensor.name if nc.partition_id_tensor else None
    in_names, out_names, out_avals = [], [], []
    for alloc in nc.m.functions[0].allocations:
        if not isinstance(alloc, mybir.MemoryLocationSet):
            continue
        name = alloc.memorylocations[0].name
        if alloc.kind == "ExternalInput":
            if name != part_name:
                in_names.append(name)
        elif alloc.kind == "ExternalOutput":
            out_names.append(name)
            shape = tuple(alloc.tensor_shape)
            out_avals.append(jax.core.ShapedArray(shape, mybir.dt.np(alloc.dtype)))
    n_params = len(in_names)
    all_names = in_names + out_names + ([part_name] if part_name else [])

    def _body(*args):
        operands = list(args)
        if part_name is not None:
            operands.append(partition_id_tensor())
        outs = _bass_exec_p.bind(
            *operands, out_avals=tuple(out_avals), in_names=tuple(all_names),
            out_names=tuple(out_names), lowering_input_output_aliases=(),
            sim_require_finite=False, sim_require_nnan=False, nc=nc)
        return tuple(outs)

    devices = jax.devices()[:ncores]
    mesh = Mesh(np.asarray(devices), ("core",))
    sharded = jax.jit(
        shard_map(_body, mesh=mesh,
                  in_specs=(PartitionSpec("core"),) * (n_params + len(out_avals)),
                  out_specs=(PartitionSpec("core"),) * len(out_avals),
                  check_rep=False),
        donate_argnums=tuple(range(n_params, n_params + len(out_avals))),
        keep_unused=True)

    def run(in_maps):
        concat_in = [np.concatenate([np.asarray(m[nm]) for m in in_maps], axis=0)
                     for nm in in_names]
        zeros = [np.zeros((ncores * a.shape[0], *a.shape[1:]), a.dtype)
                 for a in out_avals]
        out_arrs = sharded(*concat_in, *zeros)
        return {nm: np.asarray(out_arrs[i]) for i, nm in enumerate(out_names)}

    return run


def _in_maps(inputs):
    xt, consts = _host_prep(inputs)
    maps = []
    for c in range(NCORES):
        m = dict(consts)
        m["xt"] = np.ascontiguousarray(xt[:, c * BL:(c + 1) * BL])
        maps.append(m)
    return maps


def _run(inputs):
    global _PROGRAM, _RUNNER
    if _RUNNER is None:
        if _PROGRAM is None:
            _PROGRAM = _build_program()
            _legalize_waits(_PROGRAM)
        _RUNNER = _make_runner(_PROGRAM)
    outs = _RUNNER(_in_maps(inputs))
    return outs["out"]


def kernel(**inputs):
    return _run(inputs)
